# revision 20
# baseline (speedup 1.0000x reference)
"""Trainium2 Bass kernel for nn_DiscretisedBNF (discretised BNF loss).

Math reduction used on device: the reference's (B, D, K=128) clamped-CDF
bin sum collapses (Abel summation) to

    pO[b,d] = -127/256 + sum_{k=1..127} u_k * erf(z_k),
    z_k = (e_k - mu_x) * inv,   e_k = 2k/128 - 1,
    u_k = -1/128 (k<127),  u_127 = 125/256,
    inv = 1 / (sigma_x * sqrt(2))

erf is approximated on device by tanh(1.20331*z) (minimax fit, max abs
err 0.019; end-to-end loss rel err ~1.6e-3 incl. all quantization) so
that the whole kernel uses a single ACT table set (exp_and_others has
exp, tanh and leaky_relu; erf would force a ~2.7us table switch between
the exp and the binning phase).

Sharding (8 cores, full inputs in, full output out):
  - mm1 (mu_cat @ W1) replicated per core, fp8 DoubleRow (2 k-subtiles
    per matmul), with the t-row and b1 folded in as a K=2 bf16 matmul,
  - W2 column-sharded: core i owns output columns {i*128..} (mu_eps)
    and {1024+i*128..} (ln_sigma); mm2 fp8 DoubleRow + b2 ones-row,
  - binning data-parallel over the same d-slice: 32768 elements/core,
  - per-core output: 128 partial sums of sigma1^{-2t}*(x-pO)^2; host
    reduces and scales.

Inputs are host-packed into a few large SBUF-layout blobs so the input
pipe is ~15 large DMAs split across both HWDGE queues instead of ~40
small serialized ones. Element order for binning is dh-major
(g = dh*16384 + p64*256 + b, d_local = dh*64 + p64) so each half of the
prep (driven by one mu_eps/ln_sig half of mm2) feeds a contiguous run
of bin groups, letting ACT start tanh right behind mm2.
"""

import sys

sys.path.insert(0, "/opt/trn_rl_repo")

import numpy as np
import ml_dtypes

import concourse.bass as bass
import concourse.tile as tile
from concourse import bacc, mybir
from concourse.alu_op_type import AluOpType
from concourse.bass_utils import run_bass_kernel_spmd

B, D, H, K = 256, 1024, 2048, 128
NCORES = 8
DSL = D // NCORES  # 128 d-columns per core
SIGMA1 = 0.02
TMIN = 1e-10
LEAK = 0.01
C127 = 127.0 / 256.0
ATAN = 1.2033141525242548  # tanh(ATAN*z) ~= erf(z)

F32 = mybir.dt.float32
BF16 = mybir.dt.bfloat16
FP8 = mybir.dt.float8e4
BFNP = ml_dtypes.bfloat16
F8NP = ml_dtypes.float8_e4m3

HELEMS = DSL // 2 * B          # 16384 elements per dh half
RHEAD = 36 * B                 # 9216 = 6 groups of 1536 (partitions 0:36)
GROUPS = [1536] * 10 + [1024]  # per-half group sizes (sum = 16384)

# bb blob column offsets (bf16, 4 partitions)
BB_TV = 0         # [0:2, 0:256]   row0 = t, row1 = ones
BB_W1T = 256      # [0:2, 256:2304] row0 = W1[D,:], row1 = b1
BB_EDG = 2304     # [0:4, 2304:2432] edge matrix
BB_B2 = 2432      # [0:1, 2432:2688] b2[cols]
BB_ONE = 2688     # [0:1, 2688:2944] ones
BB_W = 2944

# f64 blob column offsets (f32, 64 partitions; 512-wide = tiled x2 over dh)
FO_MF, FO_BV, FO_RM, FO_CE, FO_XS, FO_NS = 0, 512, 1024, 1536, 2048, 2560
F64_W = 3072


def _build(debug=False):
    nc = bacc.Bacc("TRN2", target_bir_lowering=False, debug=False,
                   num_devices=NCORES)

    d_muT = nc.dram_tensor("muT8", (128, 8 * B), FP8, kind="ExternalInput")
    d_w1 = nc.dram_tensor("w1m", (128, 16 * 8 * 128), FP8,
                          kind="ExternalInput")
    d_w2 = nc.dram_tensor("w2m", (128, 16 * 2 * DSL), FP8,
                          kind="ExternalInput")
    d_bb = nc.dram_tensor("bb", (4, BB_W), BF16, kind="ExternalInput")
    d_f64 = nc.dram_tensor("f64", (64, F64_W), F32, kind="ExternalInput")
    d_f128 = nc.dram_tensor("f128", (128, 2 * B), F32, kind="ExternalInput")
    d_uv = nc.dram_tensor("uv", (128, 1), BF16, kind="ExternalInput")
    d_edg3 = nc.dram_tensor("edg3", (68, 128), BF16, kind="ExternalInput")
    d_part = nc.dram_tensor("part", (128, 1), F32, kind="ExternalOutput")

    MULT, ADD, SUB, BYP = (AluOpType.mult, AluOpType.add,
                           AluOpType.subtract, AluOpType.bypass)
    AF = mybir.ActivationFunctionType
    DR = mybir.MatmulPerfMode.DoubleRow

    with tile.TileContext(nc) as tc:
        with (
            tc.tile_pool(name="weights", bufs=1) as wpool,
            tc.tile_pool(name="work", bufs=1) as work,
            tc.tile_pool(name="stage", bufs=1) as stage,
        ):
            muT = wpool.tile([128, 8, B], FP8)
            w1s = [wpool.tile([128, 4, 8, 128], FP8, name=f"w1s{i}")
                   for i in range(4)]
            w2 = wpool.tile([128, 16, 2 * DSL], FP8)
            bb = wpool.tile([4, BB_W], BF16)
            f64 = wpool.tile([64, F64_W], F32)
            f128 = wpool.tile([128, 2 * B], F32)
            uv = wpool.tile([128, 1], BF16)
            edg3 = wpool.tile([68, 128], BF16)
            hT = work.tile([128, 16, B], FP8)

            with (
                tc.tile_pool(name="psA", bufs=3,
                             space=bass.MemorySpace.PSUM) as psA,
                tc.tile_pool(name="psO", bufs=1,
                             space=bass.MemorySpace.PSUM) as psO,
            ):
                # ---- input DMAs: sync (HWDGE) carries the mm1-critical
                # tensors; gpsimd (SWDGE) carries the rest. scalar/ACT
                # stays DMA-free so activations never queue behind DMAs.
                nc.sync.dma_start(bb[:], d_bb.ap()[:])
                nc.sync.dma_start(muT[:], d_muT.ap()[:])
                for s4 in range(4):  # 4 m-tiles (512KB) per slab
                    nc.sync.dma_start(
                        w1s[s4][:], d_w1.ap()[:, s4 * 4096:(s4 + 1) * 4096])
                nc.sync.dma_start(f64[:], d_f64.ap()[:])
                nc.sync.dma_start(uv[:], d_uv.ap()[:])
                nc.sync.dma_start(edg3[:], d_edg3.ap()[:])
                nc.sync.dma_start(w2[:], d_w2.ap()[:])
                nc.sync.dma_start(f128[:], d_f128.ap()[:])

                # s = x*mf + (1-gamma)*mf*noise  (the mu/gamma term, masked)
                a1 = work.tile([64, 2, B], F32)
                nc.vector.tensor_tensor(
                    a1[:], f64[:, FO_XS:FO_XS + 512], f64[:, FO_MF:FO_MF + 512],
                    MULT)
                a2 = work.tile([64, 2, B], F32)
                nc.vector.tensor_tensor(
                    a2[:], f64[:, FO_NS:FO_NS + 512], f64[:, FO_BV:FO_BV + 512],
                    MULT)
                s = work.tile([64, 2, B], F32)
                nc.vector.tensor_tensor(s[:], a1[:], a2[:], ADD)
                # dummy exp: pull the exp_and_others ACT table load into
                # the mm1 window (tanh/exp later need no load)
                dum = work.tile([1, 1], F32)
                nc.scalar.activation(dum[:], bb[0:1, 0:1], AF.Exp,
                                     bias=0.0, scale=1.0)

                # ---- mm1: hT[m] = LeakyReLU(W1^T mu_cat^T) fp8 DoubleRow;
                # t-row and b1 folded in as a K=2 bf16 matmul
                for m in range(16):
                    ph = psA.tile([128, B], F32, tag="ph")
                    for j in range(4):
                        nc.tensor.matmul(
                            ph[:], w1s[m // 4][:, m % 4, 2 * j:2 * j + 2, :],
                            muT[:, 2 * j:2 * j + 2, :],
                            start=(j == 0), stop=False, perf_mode=DR)
                    ms = slice(BB_W1T + m * 128, BB_W1T + (m + 1) * 128)
                    nc.tensor.matmul(ph[:], bb[0:2, ms], bb[0:2, BB_TV:BB_TV + B],
                                     start=False, stop=True)
                    u = work.tile([128, B], F32, tag="lrelu_u", bufs=2)
                    nc.vector.tensor_copy(u[:], ph[:])
                    nc.vector.scalar_tensor_tensor(
                        hT[:, m, :], u[:], LEAK, u[:],
                        op0=MULT, op1=AluOpType.max)

                # ---- mm2: po[mo] = W2[:,cols]^T hT + b2, fp8 DoubleRow.
                # Emitted in two halves: tiles (2,0) -> prep half a ->
                # tiles (3,1) -> prep half b. The deferred tiles give the
                # PE real work during half-a's flatten DMAs, so it never
                # idles long enough for HAM to re-throttle the clock.
                po = {}

                def mm2_tile(mo):
                    pt = psO.tile([64, B], F32, tag=f"po{mo}")
                    po[mo] = pt
                    mos = slice(mo * 64, (mo + 1) * 64)
                    for j in range(8):
                        nc.tensor.matmul(pt[:], w2[:, 2 * j:2 * j + 2, mos],
                                         hT[:, 2 * j:2 * j + 2, :],
                                         start=(j == 0), stop=False,
                                         perf_mode=DR)
                    b2s = slice(BB_B2 + mo * 64, BB_B2 + (mo + 1) * 64)
                    nc.tensor.matmul(pt[:], bb[0:1, b2s],
                                     bb[0:1, BB_ONE:BB_ONE + B],
                                     start=False, stop=True)

                # ---- binning prep, per dh half -------------------------
                QT = [stage.tile([64, 4, B], BF16, name=f"QT{h}")
                      for h in range(2)]
                R1 = stage.tile([4, RHEAD], BF16, name="R1a")
                R2 = stage.tile([4, HELEMS - RHEAD], BF16, name="R2a")
                RAb = stage.tile([4, HELEMS], BF16, name="RAb")
                R3b = stage.tile([68, 11 * 512], BF16, name="R3b")
                R3a = stage.tile([68, 5 * 512], BF16, name="R3a")
                def flatten_row(hh, r):
                    if hh == 0:
                        # half a: head (36 partitions = 6 groups) first
                        # for an early z start, then the main piece
                        nc.sync.dma_start(R1[r:r + 1, :], QT[0][0:36, r, :])
                    else:
                        nc.sync.dma_start(RAb[r:r + 1, :], QT[1][0:64, r, :])

                def flatten_main(hh, r):
                    if hh == 0:
                        nc.sync.dma_start(R2[r:r + 1, :], QT[0][36:64, r, :])

                def flatten_scatter(hh):
                    # late-consumed pieces are scattered into per-row-group
                    # copies so their z matmuls run 3x packed: half a's
                    # main piece (walked last) and all of half b
                    if hh == 0:
                        rv = R2[:].rearrange("p (blk i) -> p blk i", i=512)
                        for h in range(3):
                            nblk = 4 if h == 2 else 5
                            nc.sync.dma_start(
                                R3a[32 * h:32 * h + 4, 0:nblk * 512],
                                rv[:, h:14:3, :])
                        return
                    rav = RAb[:].rearrange("p (blk i) -> p blk i", i=512)
                    for h in range(3):
                        nblk = 10 if h == 2 else 11
                        nc.sync.dma_start(
                            R3b[32 * h:32 * h + 4, 0:nblk * 512],
                            rav[:, h:32:3, :])

                warmn = [0]

                def warm_mm(dep_q, dep_r):
                    w = psA.tile([128, B], F32, tag="warm", bufs=1)
                    nc.tensor.matmul(w[:], QT[dep_q][:, dep_r, 0:128],
                                     QT[dep_q][:, dep_r, :],
                                     start=True, stop=True)
                    warmn[0] += 1

                inv = [None, None]

                def prep_half(hh):
                    lnm = work.tile([64, B], F32, tag=f"lnm{hh}")
                    nc.vector.tensor_tensor(lnm[:], po[2 + hh][:],
                                            f64[:, FO_MF + hh * B:FO_MF + hh * B + B],
                                            MULT)
                    ei = work.tile([64, B], F32, tag=f"ei{hh}")
                    nc.scalar.activation(ei[:], lnm[:], AF.Exp, bias=0.0,
                                         scale=-1.0)
                    iv = work.tile([64, B], F32, tag=f"inv{hh}")
                    inv[hh] = iv
                    nc.vector.tensor_tensor(
                        iv[:], ei[:], f64[:, FO_CE + hh * B:FO_CE + hh * B + B],
                        MULT)
                    nc.vector.tensor_copy(QT[hh][:, 0, :], iv[:])   # ih
                    flatten_row(hh, 0)
                    nc.vector.tensor_tensor(QT[hh][:, 1, :], iv[:],
                                            QT[hh][:, 0, :], SUB)   # il
                    flatten_row(hh, 1)
                    warm_mm(hh, 0)
                    a4 = work.tile([64, B], F32, tag=f"a4{hh}")
                    nc.vector.tensor_tensor(
                        a4[:], f64[:, FO_RM + hh * B:FO_RM + hh * B + B],
                        po[hh][:], MULT)
                    mu_x = work.tile([64, B], F32, tag=f"mux{hh}")
                    nc.vector.tensor_tensor(mu_x[:], s[:, hh, :], a4[:], SUB)
                    mx = work.tile([64, B], F32, tag=f"mx{hh}")
                    nc.vector.tensor_tensor(mx[:], mu_x[:], iv[:], MULT)
                    nc.vector.tensor_copy(QT[hh][:, 2, :], mx[:])   # hi
                    flatten_row(hh, 2)
                    nc.vector.tensor_tensor(QT[hh][:, 3, :], mx[:],
                                            QT[hh][:, 2, :], SUB)   # lo
                    flatten_row(hh, 3)
                    warm_mm(hh, 2)
                    for r in range(4):
                        flatten_main(hh, r)
                    flatten_scatter(hh)

                mm2_tile(2)
                mm2_tile(0)
                prep_half(0)
                mm2_tile(3)
                mm2_tile(1)
                prep_half(1)

            # ---- binning main loop -------------------------------------
            with (
                tc.tile_pool(name="psZ", bufs=2,
                             space=bass.MemorySpace.PSUM) as psZ,
                tc.tile_pool(name="psQ", bufs=1,
                             space=bass.MemorySpace.PSUM) as psQ,
                tc.tile_pool(name="erf", bufs=3) as epool,
            ):
                q = psQ.tile([128, B], F32)
                # (hh, base, gel) walk, z matmuls emitted one group ahead
                # of tanh/q so the PE never idles waiting on ACT.
                # Order: half-a heads (unpacked, available first), then
                # half b (packed), then half a's scattered tail (packed).
                walk = []
                for hh in range(2):
                    base = 0
                    for gel in GROUPS:
                        walk.append((hh, base, gel))
                        base += gel
                walk = walk[0:6] + walk[11:22] + walk[6:11]
                zts = {}

                def emit_z(gi):
                    hh, base, gel = walk[gi]
                    zt = psZ.tile([128, 1536], F32, tag="zt")
                    zts[gi] = zt
                    if hh == 0 and base < RHEAD:
                        for h in range(gel // 512):
                            off = base + h * 512
                            nc.tensor.matmul(
                                zt[:, h * 512:(h + 1) * 512], edg3[0:4, :],
                                R1[:, off:off + 512], start=True, stop=True)
                    else:
                        src_t = R3b if hh == 1 else R3a
                        g = base // 1536 if hh == 1 else (base - RHEAD) // 1536
                        for h in range(gel // 512):
                            nc.tensor.matmul(
                                zt[:, h * 512:(h + 1) * 512],
                                edg3[32 * h:32 * h + 4, :],
                                src_t[32 * h:32 * h + 4,
                                      g * 512:(g + 1) * 512],
                                start=True, stop=True,
                                tile_position=(32 * h, 0))

                emit_z(0)
                for gi, (hh, base, gel) in enumerate(walk):
                    if gi + 1 < len(walk):
                        emit_z(gi + 1)
                    zt = zts.pop(gi)
                    et = epool.tile([128, 1536], FP8, tag="et")
                    nc.scalar.activation(et[:, 0:gel], zt[:, 0:gel],
                                         AF.Tanh, bias=0.0, scale=ATAN)
                    for j in range(gel // 128):
                        c = (hh * HELEMS + base) // 128 + j
                        nc.tensor.matmul(q[:, c:c + 1],
                                         et[:, j * 128:(j + 1) * 128],
                                         uv[:], start=True, stop=True)

                # tail: part = sum_cols (sqw*(xqc - q))^2
                e1 = work.tile([128, B], F32)
                nc.vector.tensor_tensor(e1[:], f128[:, 0:B], q[:], SUB)
                dw = work.tile([128, B], F32)
                nc.vector.tensor_tensor(dw[:], e1[:], f128[:, B:2 * B], MULT)
                dw2 = work.tile([128, B], F32)
                part = work.tile([128, 1], F32)
                nc.vector.scalar_tensor_tensor(dw2[:], dw[:], 1.0, dw[:],
                                               op0=BYP, op1=MULT,
                                               accum_out=part[:])
                nc.sync.dma_start(d_part.ap()[:], part[:])

    nc.compile()
    return nc


def host_prep(x, t, noise, W1, b1, W2, b2):
    """Build the per-core in_maps (host-side packing + tiny per-row math)."""
    f32 = np.float32
    tv = t[:, 0].astype(f32)
    gamma = (1.0 - np.power(f32(SIGMA1), f32(2.0) * tv)).astype(f32)
    low = tv < TMIN
    mf = np.where(low, f32(0.0), f32(1.0)).astype(f32)
    gsafe = np.where(gamma > 0, gamma, f32(1.0)).astype(f32)
    r = np.sqrt((1.0 - gsafe) / gsafe).astype(f32)
    rsafe = np.where(r > 0, r, f32(1.0)).astype(f32)
    bv = ((1.0 - gamma) * mf).astype(f32)
    rm = (r * mf).astype(f32)
    cexp = np.where(low, f32(1.0 / np.sqrt(2.0)),
                    (1.0 / (rsafe * np.sqrt(2.0))).astype(f32)).astype(f32)
    sqw = np.power(f32(SIGMA1), -tv).astype(f32)

    xT = np.ascontiguousarray(x.T, dtype=f32)
    nT = np.ascontiguousarray(noise.T, dtype=f32)
    g2 = (gamma * (1.0 - gamma)).astype(f32)
    muT8 = np.ascontiguousarray(
        (xT * gamma[None, :] + nT * g2[None, :]).astype(f32)
        .reshape(8, 128, B).transpose(1, 0, 2).reshape(128, 8 * B)
        .astype(F8NP))

    # w1m[p, (m*8+k)*128 + c] = W1[k*128+p, m*128+c]
    w1f = W1[:D].astype(f32).reshape(8, 128, 16, 128)
    w1m = np.ascontiguousarray(
        w1f.transpose(1, 2, 0, 3).reshape(128, 16 * 8 * 128).astype(F8NP))

    # bb blob
    bbv = np.zeros((4, BB_W), dtype=BFNP)
    bbv[0, BB_TV:BB_TV + B] = tv.astype(BFNP)
    bbv[1, BB_TV:BB_TV + B] = BFNP(1.0)
    bbv[0, BB_W1T:BB_W1T + H] = W1[D].astype(BFNP)
    bbv[1, BB_W1T:BB_W1T + H] = b1.astype(BFNP)
    e = (2.0 * np.arange(1, K) / K - 1.0).astype(f32)  # 127 edges, bf16-exact
    bbv[0, BB_ONE:BB_ONE + B] = BFNP(1.0)
    edg3 = np.zeros((68, 128), dtype=BFNP)
    for hb in (0, 32, 64):
        edg3[hb + 0, :127] = e.astype(BFNP)
        edg3[hb + 1, :127] = e.astype(BFNP)
        edg3[hb + 2, :127] = BFNP(-1.0)
        edg3[hb + 3, :127] = BFNP(-1.0)

    # f64 blob (per-batch broadcasts, tiled x2 over dh)
    f64v = np.zeros((64, F64_W), dtype=f32)
    for off, v in ((FO_MF, mf), (FO_BV, bv), (FO_RM, rm), (FO_CE, cexp)):
        f64v[:, off:off + 512] = np.tile(v, 2)[None, :]

    uvec = np.zeros((128, 1), dtype=BFNP)
    uvec[:126, 0] = BFNP(-1.0 / K)
    uvec[126, 0] = BFNP(125.0 / 256.0)  # exact in bf16

    # q layout index math: q column c = hh*128 + r, partition p.
    # half a (hh=0) is linear: elem = r*128+p -> p64 = elem//256.
    # half b went through the 3-way block scatter: chunk k = r-12g,
    # h = k//4, i = (k%4)*128+p -> p64 = (3g+h)*2 + i//256.
    p_idx = np.arange(128)[:, None]
    c_idx = np.arange(B)[None, :]
    hh = c_idx // 128
    r = c_idx % 128
    lin = r * 128 + p_idx
    p64_a = lin // B
    b_a = lin % B
    gg = np.minimum(r // 12, 10)
    k = r - gg * 12
    i = (k % 4) * 128 + p_idx
    p64_b = (3 * gg + k // 4) * 2 + i // 256
    b_b = i % 256
    # half a: heads (q cols 0..71) linear; tail (cols 72..127) scattered
    gg_a = np.minimum((r - 72) // 12, 4)
    k_a = r - 72 - gg_a * 12
    i_a = (k_a % 4) * 128 + p_idx
    p64_at = 36 + (3 * gg_a + k_a // 4) * 2 + i_a // 256
    b_at = i_a % 256
    p64 = np.where(hh == 0, np.where(r < 72, p64_a, p64_at), p64_b)
    b_i = np.where(hh == 0, np.where(r < 72, b_a, b_at), b_b)
    d_l = hh * 64 + p64
    sqwq = np.ascontiguousarray(sqw[b_i], dtype=f32)

    def to64(a128):
        # [128 d, 256 b] -> [64 p, 2, 256] with [p, dh, b] = a[dh*64+p, b]
        return np.ascontiguousarray(
            a128.reshape(2, 64, B).transpose(1, 0, 2).reshape(64, 2 * B))

    in_maps = []
    for i in range(NCORES):
        cols = np.concatenate([np.arange(i * DSL, (i + 1) * DSL),
                               1024 + np.arange(i * DSL, (i + 1) * DSL)])
        # w2m[p, k*256 + c] = W2[k*128+p, cols[c]]
        w2m = np.ascontiguousarray(
            W2[:, cols].astype(f32).reshape(16, 128, 2 * DSL)
            .transpose(1, 0, 2).reshape(128, 16 * 2 * DSL).astype(F8NP))
        bbi = bbv.copy()
        bbi[0, BB_B2:BB_B2 + 2 * DSL] = b2[cols].astype(BFNP)
        f64i = f64v.copy()
        f64i[:, FO_XS:FO_XS + 512] = to64(xT[i * DSL:(i + 1) * DSL])
        f64i[:, FO_NS:FO_NS + 512] = to64(nT[i * DSL:(i + 1) * DSL])
        f128 = np.empty((128, 2 * B), dtype=f32)
        f128[:, 0:B] = x[b_i, i * DSL + d_l].astype(f32) + f32(C127)
        f128[:, B:2 * B] = sqwq
        in_maps.append({
            "muT8": muT8, "w1m": w1m, "w2m": w2m, "bb": bbi,
            "f64": f64i, "f128": f128, "uv": uvec, "edg3": edg3,
        })
    return in_maps


_nc_cache = {}


def get_nc(debug=False):
    if debug not in _nc_cache:
        _nc_cache[debug] = _build(debug)
    return _nc_cache[debug]


def run_on_cores(inputs, trace=False, debug=False, tmpdir=None):
    nc = get_nc(debug)
    in_maps = host_prep(**inputs)
    res = run_bass_kernel_spmd(nc, in_maps, core_ids=list(range(NCORES)),
                               trace=trace, tmpdir=tmpdir)
    total = np.float32(0.0)
    for i in range(NCORES):
        total += res.results[i]["part"].astype(np.float32).sum()
    loss = np.float32(-np.log(np.float32(SIGMA1)) * total / np.float32(B * D))
    return loss, res


def kernel(**inputs):
    inputs = {k: np.asarray(v) for k, v in inputs.items()}
    loss, _ = run_on_cores(inputs)
    return np.asarray(loss, dtype=np.float32)


# revision 21
# speedup vs baseline: 1.0292x; 1.0292x over previous
"""Trainium2 Bass kernel for nn_DiscretisedBNF (discretised BNF loss).

Math reduction used on device: the reference's (B, D, K=128) clamped-CDF
bin sum collapses (Abel summation) to

    pO[b,d] = -127/256 + sum_{k=1..127} u_k * erf(z_k),
    z_k = (e_k - mu_x) * inv,   e_k = 2k/128 - 1,
    u_k = -1/128 (k<127),  u_127 = 125/256,
    inv = 1 / (sigma_x * sqrt(2))

erf is approximated on device by tanh(1.20331*z) (minimax fit, max abs
err 0.019; end-to-end loss rel err ~1.6e-3 incl. all quantization) so
that the whole kernel uses a single ACT table set (exp_and_others has
exp, tanh and leaky_relu; erf would force a ~2.7us table switch between
the exp and the binning phase).

Sharding (8 cores, full inputs in, full output out):
  - mm1 (mu_cat @ W1) replicated per core, fp8 DoubleRow (2 k-subtiles
    per matmul), with the t-row and b1 folded in as a K=2 bf16 matmul,
  - W2 column-sharded: core i owns output columns {i*128..} (mu_eps)
    and {1024+i*128..} (ln_sigma); mm2 fp8 DoubleRow + b2 ones-row,
  - binning data-parallel over the same d-slice: 32768 elements/core,
  - per-core output: 128 partial sums of sigma1^{-2t}*(x-pO)^2; host
    reduces and scales.

Inputs are host-packed into a few large SBUF-layout blobs so the input
pipe is ~15 large DMAs split across both HWDGE queues instead of ~40
small serialized ones. Element order for binning is dh-major
(g = dh*16384 + p64*256 + b, d_local = dh*64 + p64) so each half of the
prep (driven by one mu_eps/ln_sig half of mm2) feeds a contiguous run
of bin groups, letting ACT start tanh right behind mm2.
"""

import sys

sys.path.insert(0, "/opt/trn_rl_repo")

import numpy as np
import ml_dtypes

import concourse.bass as bass
import concourse.tile as tile
from concourse import bacc, mybir
from concourse.alu_op_type import AluOpType
from concourse.bass_utils import run_bass_kernel_spmd

B, D, H, K = 256, 1024, 2048, 128
NCORES = 8
DSL = D // NCORES  # 128 d-columns per core
SIGMA1 = 0.02
TMIN = 1e-10
LEAK = 0.01
C127 = 127.0 / 256.0
ATAN = 1.2033141525242548  # tanh(ATAN*z) ~= erf(z)

F32 = mybir.dt.float32
BF16 = mybir.dt.bfloat16
FP8 = mybir.dt.float8e4
BFNP = ml_dtypes.bfloat16
F8NP = ml_dtypes.float8_e4m3

HELEMS = DSL // 2 * B          # 16384 elements per dh half
RHEAD = 36 * B                 # 9216 = 6 groups of 1536 (partitions 0:36)
GROUPS = [1536] * 10 + [1024]  # per-half group sizes (sum = 16384)

# bb blob column offsets (bf16, 4 partitions)
BB_TV = 0         # [0:2, 0:256]   row0 = t, row1 = ones
BB_W1T = 256      # [0:2, 256:2304] row0 = W1[D,:], row1 = b1
BB_EDG = 2304     # [0:4, 2304:2432] edge matrix
BB_B2 = 2432      # [0:1, 2432:2688] b2[cols]
BB_ONE = 2688     # [0:1, 2688:2944] ones
BB_W = 2944

# f64 blob column offsets (f32, 64 partitions; 512-wide = tiled x2 over dh)
FO_MF, FO_BV, FO_RM, FO_CE, FO_XS, FO_NS = 0, 512, 1024, 1536, 2048, 2560
F64_W = 3072


def _build(debug=False):
    nc = bacc.Bacc("TRN2", target_bir_lowering=False, debug=False,
                   num_devices=NCORES)

    d_muT = nc.dram_tensor("muT8", (128, 8 * B), FP8, kind="ExternalInput")
    d_w1 = nc.dram_tensor("w1m", (128, 16 * 8 * 128), FP8,
                          kind="ExternalInput")
    d_w2 = nc.dram_tensor("w2m", (128, 16 * 2 * DSL), FP8,
                          kind="ExternalInput")
    d_bb = nc.dram_tensor("bb", (4, BB_W), BF16, kind="ExternalInput")
    d_f64 = nc.dram_tensor("f64", (64, F64_W), F32, kind="ExternalInput")
    d_f128 = nc.dram_tensor("f128", (128, 2 * B), F32, kind="ExternalInput")
    d_uv = nc.dram_tensor("uv", (128, 1), BF16, kind="ExternalInput")
    d_edg3 = nc.dram_tensor("edg3", (68, 128), BF16, kind="ExternalInput")
    d_part = nc.dram_tensor("part", (128, 1), F32, kind="ExternalOutput")

    MULT, ADD, SUB, BYP = (AluOpType.mult, AluOpType.add,
                           AluOpType.subtract, AluOpType.bypass)
    AF = mybir.ActivationFunctionType
    DR = mybir.MatmulPerfMode.DoubleRow

    with tile.TileContext(nc) as tc:
        with (
            tc.tile_pool(name="weights", bufs=1) as wpool,
            tc.tile_pool(name="work", bufs=1) as work,
            tc.tile_pool(name="stage", bufs=1) as stage,
        ):
            muT = wpool.tile([128, 8, B], FP8)
            w1s = [wpool.tile([128, 4, 8, 128], FP8, name=f"w1s{i}")
                   for i in range(4)]
            w2 = wpool.tile([128, 16, 2 * DSL], FP8)
            bb = wpool.tile([4, BB_W], BF16)
            f64 = wpool.tile([64, F64_W], F32)
            f128 = wpool.tile([128, 2 * B], F32)
            uv = wpool.tile([128, 1], BF16)
            edg3 = wpool.tile([68, 128], BF16)
            hT = work.tile([128, 16, B], FP8)

            with (
                tc.tile_pool(name="psA", bufs=3,
                             space=bass.MemorySpace.PSUM) as psA,
                tc.tile_pool(name="psO", bufs=1,
                             space=bass.MemorySpace.PSUM) as psO,
            ):
                # ---- input DMAs: sync (HWDGE) carries the mm1-critical
                # tensors; gpsimd (SWDGE) carries the rest. scalar/ACT
                # stays DMA-free so activations never queue behind DMAs.
                nc.sync.dma_start(bb[:], d_bb.ap()[:])
                nc.sync.dma_start(muT[:], d_muT.ap()[:])
                for s4 in range(4):  # 4 m-tiles (512KB) per slab
                    nc.sync.dma_start(
                        w1s[s4][:], d_w1.ap()[:, s4 * 4096:(s4 + 1) * 4096])
                nc.sync.dma_start(f64[:], d_f64.ap()[:])
                nc.sync.dma_start(uv[:], d_uv.ap()[:])
                nc.sync.dma_start(edg3[:], d_edg3.ap()[:])
                nc.sync.dma_start(w2[:], d_w2.ap()[:])
                nc.sync.dma_start(f128[:], d_f128.ap()[:])

                # s = x*mf + (1-gamma)*mf*noise  (the mu/gamma term, masked)
                a1 = work.tile([64, 2, B], F32)
                nc.vector.tensor_tensor(
                    a1[:], f64[:, FO_XS:FO_XS + 512], f64[:, FO_MF:FO_MF + 512],
                    MULT)
                a2 = work.tile([64, 2, B], F32)
                nc.vector.tensor_tensor(
                    a2[:], f64[:, FO_NS:FO_NS + 512], f64[:, FO_BV:FO_BV + 512],
                    MULT)
                s = work.tile([64, 2, B], F32)
                nc.vector.tensor_tensor(s[:], a1[:], a2[:], ADD)
                # dummy exp: pull the exp_and_others ACT table load into
                # the mm1 window (tanh/exp later need no load)
                dum = work.tile([1, 1], F32)
                nc.scalar.activation(dum[:], bb[0:1, 0:1], AF.Exp,
                                     bias=0.0, scale=1.0)

                # ---- mm1: hT[m] = LeakyReLU(W1^T mu_cat^T) fp8 DoubleRow;
                # t-row and b1 folded in as a K=2 bf16 matmul
                for m in range(16):
                    ph = psA.tile([128, B], F32, tag="ph")
                    for j in range(4):
                        nc.tensor.matmul(
                            ph[:], w1s[m // 4][:, m % 4, 2 * j:2 * j + 2, :],
                            muT[:, 2 * j:2 * j + 2, :],
                            start=(j == 0), stop=False, perf_mode=DR)
                    ms = slice(BB_W1T + m * 128, BB_W1T + (m + 1) * 128)
                    nc.tensor.matmul(ph[:], bb[0:2, ms], bb[0:2, BB_TV:BB_TV + B],
                                     start=False, stop=True)
                    u = work.tile([128, B], F32, tag="lrelu_u", bufs=2)
                    nc.vector.tensor_copy(u[:], ph[:])
                    nc.vector.scalar_tensor_tensor(
                        hT[:, m, :], u[:], LEAK, u[:],
                        op0=MULT, op1=AluOpType.max)

                # ---- mm2: po[mo] = W2[:,cols]^T hT + b2, fp8 DoubleRow.
                # Emitted in two halves: tiles (2,0) -> prep half a ->
                # tiles (3,1) -> prep half b. The deferred tiles give the
                # PE real work during half-a's flatten DMAs, so it never
                # idles long enough for HAM to re-throttle the clock.
                po = {}

                def mm2_tile(mo):
                    pt = psO.tile([64, B], F32, tag=f"po{mo}")
                    po[mo] = pt
                    mos = slice(mo * 64, (mo + 1) * 64)
                    for j in range(8):
                        nc.tensor.matmul(pt[:], w2[:, 2 * j:2 * j + 2, mos],
                                         hT[:, 2 * j:2 * j + 2, :],
                                         start=(j == 0), stop=False,
                                         perf_mode=DR)
                    b2s = slice(BB_B2 + mo * 64, BB_B2 + (mo + 1) * 64)
                    nc.tensor.matmul(pt[:], bb[0:1, b2s],
                                     bb[0:1, BB_ONE:BB_ONE + B],
                                     start=False, stop=True)

                # ---- binning prep, per dh half -------------------------
                QT = [stage.tile([64, 4, B], BF16, name=f"QT{h}")
                      for h in range(2)]
                R1 = stage.tile([4, RHEAD], BF16, name="R1a")
                R2 = stage.tile([4, HELEMS - RHEAD], BF16, name="R2a")
                RAb = stage.tile([4, HELEMS], BF16, name="RAb")
                R3b = stage.tile([68, 11 * 512], BF16, name="R3b")
                R3a = stage.tile([68, 5 * 512], BF16, name="R3a")
                def flatten_row(hh, r):
                    if hh == 0:
                        # half a: head (36 partitions = 6 groups) first
                        # for an early z start, then the main piece
                        nc.sync.dma_start(R1[r:r + 1, :], QT[0][0:36, r, :])
                    else:
                        nc.sync.dma_start(RAb[r:r + 1, :], QT[1][0:64, r, :])

                def flatten_main(hh, r):
                    if hh == 0:
                        nc.sync.dma_start(R2[r:r + 1, :], QT[0][36:64, r, :])

                def flatten_scatter(hh):
                    # late-consumed pieces are scattered into per-row-group
                    # copies so their z matmuls run 3x packed: half a's
                    # main piece (walked last) and all of half b
                    if hh == 0:
                        rv = R2[:].rearrange("p (blk i) -> p blk i", i=512)
                        for h in range(3):
                            nblk = 4 if h == 2 else 5
                            nc.sync.dma_start(
                                R3a[32 * h:32 * h + 4, 0:nblk * 512],
                                rv[:, h:14:3, :])
                        return
                    rav = RAb[:].rearrange("p (blk i) -> p blk i", i=512)
                    for h in range(3):
                        nblk = 10 if h == 2 else 11
                        nc.sync.dma_start(
                            R3b[32 * h:32 * h + 4, 0:nblk * 512],
                            rav[:, h:32:3, :])

                warmn = [0]

                def warm_mm(dep_q, dep_r):
                    w = psA.tile([128, B], F32, tag="warm", bufs=1)
                    nc.tensor.matmul(w[:], QT[dep_q][:, dep_r, 0:128],
                                     QT[dep_q][:, dep_r, :],
                                     start=True, stop=True)
                    warmn[0] += 1

                inv = [None, None]

                def prep_half(hh):
                    lnm = work.tile([64, B], F32, tag=f"lnm{hh}")
                    nc.vector.tensor_tensor(lnm[:], po[2 + hh][:],
                                            f64[:, FO_MF + hh * B:FO_MF + hh * B + B],
                                            MULT)
                    ei = work.tile([64, B], F32, tag=f"ei{hh}")
                    nc.scalar.activation(ei[:], lnm[:], AF.Exp, bias=0.0,
                                         scale=-1.0)
                    iv = work.tile([64, B], F32, tag=f"inv{hh}")
                    inv[hh] = iv
                    nc.vector.tensor_tensor(
                        iv[:], ei[:], f64[:, FO_CE + hh * B:FO_CE + hh * B + B],
                        MULT)
                    nc.vector.tensor_copy(QT[hh][:, 0, :], iv[:])   # ih
                    flatten_row(hh, 0)
                    nc.vector.tensor_tensor(QT[hh][:, 1, :], iv[:],
                                            QT[hh][:, 0, :], SUB)   # il
                    flatten_row(hh, 1)
                    warm_mm(hh, 0)
                    a4 = work.tile([64, B], F32, tag=f"a4{hh}")
                    nc.vector.tensor_tensor(
                        a4[:], f64[:, FO_RM + hh * B:FO_RM + hh * B + B],
                        po[hh][:], MULT)
                    mu_x = work.tile([64, B], F32, tag=f"mux{hh}")
                    nc.vector.tensor_tensor(mu_x[:], s[:, hh, :], a4[:], SUB)
                    mx = work.tile([64, B], F32, tag=f"mx{hh}")
                    nc.vector.tensor_tensor(mx[:], mu_x[:], iv[:], MULT)
                    nc.vector.tensor_copy(QT[hh][:, 2, :], mx[:])   # hi
                    flatten_row(hh, 2)
                    nc.vector.tensor_tensor(QT[hh][:, 3, :], mx[:],
                                            QT[hh][:, 2, :], SUB)   # lo
                    flatten_row(hh, 3)
                    warm_mm(hh, 2)
                    for r in range(4):
                        flatten_main(hh, r)
                    flatten_scatter(hh)

                mm2_tile(2)
                mm2_tile(0)
                prep_half(0)
                mm2_tile(3)
                mm2_tile(1)
                prep_half(1)

            # ---- binning main loop -------------------------------------
            with (
                tc.tile_pool(name="psZ", bufs=2,
                             space=bass.MemorySpace.PSUM) as psZ,
                tc.tile_pool(name="psQ", bufs=1,
                             space=bass.MemorySpace.PSUM) as psQ,
                tc.tile_pool(name="erf", bufs=3) as epool,
            ):
                q = psQ.tile([128, B], F32)
                # (hh, base, gel) walk, z matmuls emitted one group ahead
                # of tanh/q so the PE never idles waiting on ACT.
                # Order: half-a heads (unpacked, available first), then
                # half b (packed), then half a's scattered tail (packed).
                walk = []
                for hh in range(2):
                    base = 0
                    for gel in GROUPS:
                        walk.append((hh, base, gel))
                        base += gel

                zts = {}

                def emit_z(gi):
                    hh, base, gel = walk[gi]
                    zt = psZ.tile([128, 1536], F32, tag="zt")
                    zts[gi] = zt
                    if hh == 0 and base < RHEAD:
                        for h in range(gel // 512):
                            off = base + h * 512
                            nc.tensor.matmul(
                                zt[:, h * 512:(h + 1) * 512], edg3[0:4, :],
                                R1[:, off:off + 512], start=True, stop=True)
                    else:
                        src_t = R3b if hh == 1 else R3a
                        g = base // 1536 if hh == 1 else (base - RHEAD) // 1536
                        for h in range(gel // 512):
                            nc.tensor.matmul(
                                zt[:, h * 512:(h + 1) * 512],
                                edg3[32 * h:32 * h + 4, :],
                                src_t[32 * h:32 * h + 4,
                                      g * 512:(g + 1) * 512],
                                start=True, stop=True,
                                tile_position=(32 * h, 0))

                emit_z(0)
                for gi, (hh, base, gel) in enumerate(walk):
                    if gi + 1 < len(walk):
                        emit_z(gi + 1)
                    zt = zts.pop(gi)
                    et = epool.tile([128, 1536], FP8, tag="et")
                    nc.scalar.activation(et[:, 0:gel], zt[:, 0:gel],
                                         AF.Tanh, bias=0.0, scale=ATAN)
                    for j in range(gel // 128):
                        c = (hh * HELEMS + base) // 128 + j
                        nc.tensor.matmul(q[:, c:c + 1],
                                         et[:, j * 128:(j + 1) * 128],
                                         uv[:], start=True, stop=True)

                # tail: part = sum_cols (sqw*(xqc - q))^2
                e1 = work.tile([128, B], F32)
                nc.vector.tensor_tensor(e1[:], f128[:, 0:B], q[:], SUB)
                dw = work.tile([128, B], F32)
                nc.vector.tensor_tensor(dw[:], e1[:], f128[:, B:2 * B], MULT)
                dw2 = work.tile([128, B], F32)
                part = work.tile([128, 1], F32)
                nc.vector.scalar_tensor_tensor(dw2[:], dw[:], 1.0, dw[:],
                                               op0=BYP, op1=MULT,
                                               accum_out=part[:])
                nc.sync.dma_start(d_part.ap()[:], part[:])

    nc.compile()
    return nc


def host_prep(x, t, noise, W1, b1, W2, b2):
    """Build the per-core in_maps (host-side packing + tiny per-row math)."""
    f32 = np.float32
    tv = t[:, 0].astype(f32)
    gamma = (1.0 - np.power(f32(SIGMA1), f32(2.0) * tv)).astype(f32)
    low = tv < TMIN
    mf = np.where(low, f32(0.0), f32(1.0)).astype(f32)
    gsafe = np.where(gamma > 0, gamma, f32(1.0)).astype(f32)
    r = np.sqrt((1.0 - gsafe) / gsafe).astype(f32)
    rsafe = np.where(r > 0, r, f32(1.0)).astype(f32)
    bv = ((1.0 - gamma) * mf).astype(f32)
    rm = (r * mf).astype(f32)
    cexp = np.where(low, f32(1.0 / np.sqrt(2.0)),
                    (1.0 / (rsafe * np.sqrt(2.0))).astype(f32)).astype(f32)
    sqw = np.power(f32(SIGMA1), -tv).astype(f32)

    xT = np.ascontiguousarray(x.T, dtype=f32)
    nT = np.ascontiguousarray(noise.T, dtype=f32)
    g2 = (gamma * (1.0 - gamma)).astype(f32)
    muT8 = np.ascontiguousarray(
        (xT * gamma[None, :] + nT * g2[None, :]).astype(f32)
        .reshape(8, 128, B).transpose(1, 0, 2).reshape(128, 8 * B)
        .astype(F8NP))

    # w1m[p, (m*8+k)*128 + c] = W1[k*128+p, m*128+c]
    w1f = W1[:D].astype(f32).reshape(8, 128, 16, 128)
    w1m = np.ascontiguousarray(
        w1f.transpose(1, 2, 0, 3).reshape(128, 16 * 8 * 128).astype(F8NP))

    # bb blob
    bbv = np.zeros((4, BB_W), dtype=BFNP)
    bbv[0, BB_TV:BB_TV + B] = tv.astype(BFNP)
    bbv[1, BB_TV:BB_TV + B] = BFNP(1.0)
    bbv[0, BB_W1T:BB_W1T + H] = W1[D].astype(BFNP)
    bbv[1, BB_W1T:BB_W1T + H] = b1.astype(BFNP)
    e = (2.0 * np.arange(1, K) / K - 1.0).astype(f32)  # 127 edges, bf16-exact
    bbv[0, BB_ONE:BB_ONE + B] = BFNP(1.0)
    edg3 = np.zeros((68, 128), dtype=BFNP)
    for hb in (0, 32, 64):
        edg3[hb + 0, :127] = e.astype(BFNP)
        edg3[hb + 1, :127] = e.astype(BFNP)
        edg3[hb + 2, :127] = BFNP(-1.0)
        edg3[hb + 3, :127] = BFNP(-1.0)

    # f64 blob (per-batch broadcasts, tiled x2 over dh)
    f64v = np.zeros((64, F64_W), dtype=f32)
    for off, v in ((FO_MF, mf), (FO_BV, bv), (FO_RM, rm), (FO_CE, cexp)):
        f64v[:, off:off + 512] = np.tile(v, 2)[None, :]

    uvec = np.zeros((128, 1), dtype=BFNP)
    uvec[:126, 0] = BFNP(-1.0 / K)
    uvec[126, 0] = BFNP(125.0 / 256.0)  # exact in bf16

    # q layout index math: q column c = hh*128 + r, partition p.
    # half a (hh=0) is linear: elem = r*128+p -> p64 = elem//256.
    # half b went through the 3-way block scatter: chunk k = r-12g,
    # h = k//4, i = (k%4)*128+p -> p64 = (3g+h)*2 + i//256.
    p_idx = np.arange(128)[:, None]
    c_idx = np.arange(B)[None, :]
    hh = c_idx // 128
    r = c_idx % 128
    lin = r * 128 + p_idx
    p64_a = lin // B
    b_a = lin % B
    gg = np.minimum(r // 12, 10)
    k = r - gg * 12
    i = (k % 4) * 128 + p_idx
    p64_b = (3 * gg + k // 4) * 2 + i // 256
    b_b = i % 256
    # half a: heads (q cols 0..71) linear; tail (cols 72..127) scattered
    gg_a = np.minimum((r - 72) // 12, 4)
    k_a = r - 72 - gg_a * 12
    i_a = (k_a % 4) * 128 + p_idx
    p64_at = 36 + (3 * gg_a + k_a // 4) * 2 + i_a // 256
    b_at = i_a % 256
    p64 = np.where(hh == 0, np.where(r < 72, p64_a, p64_at), p64_b)
    b_i = np.where(hh == 0, np.where(r < 72, b_a, b_at), b_b)
    d_l = hh * 64 + p64
    sqwq = np.ascontiguousarray(sqw[b_i], dtype=f32)

    def to64(a128):
        # [128 d, 256 b] -> [64 p, 2, 256] with [p, dh, b] = a[dh*64+p, b]
        return np.ascontiguousarray(
            a128.reshape(2, 64, B).transpose(1, 0, 2).reshape(64, 2 * B))

    in_maps = []
    for i in range(NCORES):
        cols = np.concatenate([np.arange(i * DSL, (i + 1) * DSL),
                               1024 + np.arange(i * DSL, (i + 1) * DSL)])
        # w2m[p, k*256 + c] = W2[k*128+p, cols[c]]
        w2m = np.ascontiguousarray(
            W2[:, cols].astype(f32).reshape(16, 128, 2 * DSL)
            .transpose(1, 0, 2).reshape(128, 16 * 2 * DSL).astype(F8NP))
        bbi = bbv.copy()
        bbi[0, BB_B2:BB_B2 + 2 * DSL] = b2[cols].astype(BFNP)
        f64i = f64v.copy()
        f64i[:, FO_XS:FO_XS + 512] = to64(xT[i * DSL:(i + 1) * DSL])
        f64i[:, FO_NS:FO_NS + 512] = to64(nT[i * DSL:(i + 1) * DSL])
        f128 = np.empty((128, 2 * B), dtype=f32)
        f128[:, 0:B] = x[b_i, i * DSL + d_l].astype(f32) + f32(C127)
        f128[:, B:2 * B] = sqwq
        in_maps.append({
            "muT8": muT8, "w1m": w1m, "w2m": w2m, "bb": bbi,
            "f64": f64i, "f128": f128, "uv": uvec, "edg3": edg3,
        })
    return in_maps


_nc_cache = {}


def get_nc(debug=False):
    if debug not in _nc_cache:
        _nc_cache[debug] = _build(debug)
    return _nc_cache[debug]


def run_on_cores(inputs, trace=False, debug=False, tmpdir=None):
    nc = get_nc(debug)
    in_maps = host_prep(**inputs)
    res = run_bass_kernel_spmd(nc, in_maps, core_ids=list(range(NCORES)),
                               trace=trace, tmpdir=tmpdir)
    total = np.float32(0.0)
    for i in range(NCORES):
        total += res.results[i]["part"].astype(np.float32).sum()
    loss = np.float32(-np.log(np.float32(SIGMA1)) * total / np.float32(B * D))
    return loss, res


def kernel(**inputs):
    inputs = {k: np.asarray(v) for k, v in inputs.items()}
    loss, _ = run_on_cores(inputs)
    return np.asarray(loss, dtype=np.float32)


# revision 22
# speedup vs baseline: 1.1431x; 1.1107x over previous
"""Trainium2 Bass kernel for nn_DiscretisedBNF (discretised BNF loss).

Math reduction used on device: the reference's (B, D, K=128) clamped-CDF
bin sum collapses (Abel summation) to

    pO[b,d] = -127/256 + sum_{k=1..127} u_k * erf(z_k),
    z_k = (e_k - mu_x) * inv,   e_k = 2k/128 - 1,
    u_k = -1/128 (k<127),  u_127 = 125/256,
    inv = 1 / (sigma_x * sqrt(2))

erf is approximated on device by tanh(1.20331*z) (minimax fit, max abs
err 0.019; end-to-end loss rel err ~1.6e-3 incl. all quantization) so
that the whole kernel uses a single ACT table set (exp_and_others has
exp, tanh and leaky_relu; erf would force a ~2.7us table switch between
the exp and the binning phase).

Sharding (8 cores, full inputs in, full output out):
  - mm1 (mu_cat @ W1) replicated per core, fp8 DoubleRow (2 k-subtiles
    per matmul), with the t-row and b1 folded in as a K=2 bf16 matmul,
  - W2 column-sharded: core i owns output columns {i*128..} (mu_eps)
    and {1024+i*128..} (ln_sigma); mm2 fp8 DoubleRow + b2 ones-row,
  - binning data-parallel over the same d-slice: 32768 elements/core,
  - per-core output: 128 partial sums of sigma1^{-2t}*(x-pO)^2; host
    reduces and scales.

Inputs are host-packed into a few large SBUF-layout blobs so the input
pipe is ~15 large DMAs split across both HWDGE queues instead of ~40
small serialized ones. Element order for binning is dh-major
(g = dh*16384 + p64*256 + b, d_local = dh*64 + p64) so each half of the
prep (driven by one mu_eps/ln_sig half of mm2) feeds a contiguous run
of bin groups, letting ACT start tanh right behind mm2.
"""

import sys

sys.path.insert(0, "/opt/trn_rl_repo")

import numpy as np
import ml_dtypes

import concourse.bass as bass
import concourse.tile as tile
from concourse import bacc, mybir
from concourse.alu_op_type import AluOpType
from concourse.bass_utils import run_bass_kernel_spmd

B, D, H, K = 256, 1024, 2048, 128
NCORES = 8
DSL = D // NCORES  # 128 d-columns per core
SIGMA1 = 0.02
TMIN = 1e-10
LEAK = 0.01
C127 = 127.0 / 256.0
ATAN = 1.2033141525242548  # tanh(ATAN*z) ~= erf(z)

F32 = mybir.dt.float32
BF16 = mybir.dt.bfloat16
FP8 = mybir.dt.float8e4
BFNP = ml_dtypes.bfloat16
F8NP = ml_dtypes.float8_e4m3

HELEMS = DSL // 2 * B          # 16384 elements per dh half
RHEAD = 36 * B                 # 9216 = 6 groups of 1536 (partitions 0:36)
GROUPS = [1536] * 10 + [1024]  # per-half group sizes (sum = 16384)

# bb blob column offsets (bf16, 4 partitions)
BB_TV = 0         # [0:2, 0:256]   row0 = t, row1 = ones
BB_W1T = 256      # [0:2, 256:2304] row0 = W1[D,:], row1 = b1
BB_EDG = 2304     # [0:4, 2304:2432] edge matrix
BB_B2 = 2432      # [0:1, 2432:2688] b2[cols]
BB_ONE = 2688     # [0:1, 2688:2944] ones
BB_W = 2944

# f64 blob column offsets (f32, 64 partitions; 512-wide = tiled x2 over dh)
FO_MF, FO_BV, FO_RM, FO_CE, FO_XS, FO_NS = 0, 512, 1024, 1536, 2048, 2560
F64_W = 3072


def _build(debug=False):
    nc = bacc.Bacc("TRN2", target_bir_lowering=False, debug=False,
                   num_devices=NCORES)

    d_muT = nc.dram_tensor("muT8", (128, 8 * B), FP8, kind="ExternalInput")
    d_w1 = nc.dram_tensor("w1m", (128, 16 * 8 * 128), FP8,
                          kind="ExternalInput")
    d_w2 = nc.dram_tensor("w2m", (128, 16 * 2 * DSL), FP8,
                          kind="ExternalInput")
    d_bb = nc.dram_tensor("bb", (4, BB_W), BF16, kind="ExternalInput")
    d_f64 = nc.dram_tensor("f64", (64, F64_W), F32, kind="ExternalInput")
    d_f128 = nc.dram_tensor("f128", (128, 2 * B), F32, kind="ExternalInput")
    d_uv = nc.dram_tensor("uv", (128, 1), BF16, kind="ExternalInput")
    d_edg3 = nc.dram_tensor("edg3", (68, 128), BF16, kind="ExternalInput")
    d_part = nc.dram_tensor("part", (128, 1), F32, kind="ExternalOutput")

    MULT, ADD, SUB, BYP = (AluOpType.mult, AluOpType.add,
                           AluOpType.subtract, AluOpType.bypass)
    AF = mybir.ActivationFunctionType
    DR = mybir.MatmulPerfMode.DoubleRow

    with tile.TileContext(nc) as tc:
        with (
            tc.tile_pool(name="weights", bufs=1) as wpool,
            tc.tile_pool(name="work", bufs=1) as work,
            tc.tile_pool(name="stage", bufs=1) as stage,
        ):
            muT = wpool.tile([128, 8, B], FP8)
            w1s = [wpool.tile([128, 4, 8, 128], FP8, name=f"w1s{i}")
                   for i in range(4)]
            w2 = wpool.tile([128, 16, 2 * DSL], FP8)
            bb = wpool.tile([4, BB_W], BF16)
            f64 = wpool.tile([64, F64_W], F32)
            f128 = wpool.tile([128, 2 * B], F32)
            uv = wpool.tile([128, 1], BF16)
            edg3 = wpool.tile([68, 128], BF16)
            hT = work.tile([128, 16, B], FP8)

            with (
                tc.tile_pool(name="psA", bufs=3,
                             space=bass.MemorySpace.PSUM) as psA,
                tc.tile_pool(name="psO", bufs=1,
                             space=bass.MemorySpace.PSUM) as psO,
            ):
                # ---- input DMAs: sync (HWDGE) carries the mm1-critical
                # tensors; gpsimd (SWDGE) carries the rest. scalar/ACT
                # stays DMA-free so activations never queue behind DMAs.
                nc.sync.dma_start(muT[:], d_muT.ap()[:])
                nc.sync.dma_start(w1s[0][:], d_w1.ap()[:, 0:4096])
                nc.sync.dma_start(bb[:], d_bb.ap()[:])
                for s4 in range(1, 4):  # 4 m-tiles (512KB) per slab
                    nc.sync.dma_start(
                        w1s[s4][:], d_w1.ap()[:, s4 * 4096:(s4 + 1) * 4096])
                nc.sync.dma_start(f64[:], d_f64.ap()[:])
                nc.sync.dma_start(uv[:], d_uv.ap()[:])
                nc.sync.dma_start(edg3[:], d_edg3.ap()[:])
                nc.sync.dma_start(w2[:], d_w2.ap()[:])
                nc.sync.dma_start(f128[:], d_f128.ap()[:])

                # s = x*mf + (1-gamma)*mf*noise  (the mu/gamma term, masked)
                a1 = work.tile([64, 2, B], F32)
                nc.vector.tensor_tensor(
                    a1[:], f64[:, FO_XS:FO_XS + 512], f64[:, FO_MF:FO_MF + 512],
                    MULT)
                a2 = work.tile([64, 2, B], F32)
                nc.vector.tensor_tensor(
                    a2[:], f64[:, FO_NS:FO_NS + 512], f64[:, FO_BV:FO_BV + 512],
                    MULT)
                s = work.tile([64, 2, B], F32)
                nc.vector.tensor_tensor(s[:], a1[:], a2[:], ADD)
                # dummy exp: pull the exp_and_others ACT table load into
                # the mm1 window (tanh/exp later need no load)
                dum = work.tile([1, 1], F32)
                nc.scalar.activation(dum[:], bb[0:1, 0:1], AF.Exp,
                                     bias=0.0, scale=1.0)

                # ---- mm1: hT[m] = LeakyReLU(W1^T mu_cat^T) fp8 DoubleRow;
                # t-row and b1 folded in as a K=2 bf16 matmul
                for m in range(16):
                    ph = psA.tile([128, B], F32, tag="ph")
                    for j in range(4):
                        nc.tensor.matmul(
                            ph[:], w1s[m // 4][:, m % 4, 2 * j:2 * j + 2, :],
                            muT[:, 2 * j:2 * j + 2, :],
                            start=(j == 0), stop=False, perf_mode=DR)
                    ms = slice(BB_W1T + m * 128, BB_W1T + (m + 1) * 128)
                    nc.tensor.matmul(ph[:], bb[0:2, ms], bb[0:2, BB_TV:BB_TV + B],
                                     start=False, stop=True)
                    u = work.tile([128, B], F32, tag="lrelu_u", bufs=2)
                    nc.vector.tensor_copy(u[:], ph[:])
                    nc.vector.scalar_tensor_tensor(
                        hT[:, m, :], u[:], LEAK, u[:],
                        op0=MULT, op1=AluOpType.max)

                # ---- mm2: po[mo] = W2[:,cols]^T hT + b2, fp8 DoubleRow.
                # Emitted in two halves: tiles (2,0) -> prep half a ->
                # tiles (3,1) -> prep half b. The deferred tiles give the
                # PE real work during half-a's flatten DMAs, so it never
                # idles long enough for HAM to re-throttle the clock.
                po = {}

                def mm2_tile(mo):
                    pt = psO.tile([64, B], F32, tag=f"po{mo}")
                    po[mo] = pt
                    mos = slice(mo * 64, (mo + 1) * 64)
                    for j in range(8):
                        nc.tensor.matmul(pt[:], w2[:, 2 * j:2 * j + 2, mos],
                                         hT[:, 2 * j:2 * j + 2, :],
                                         start=(j == 0), stop=False,
                                         perf_mode=DR)
                    b2s = slice(BB_B2 + mo * 64, BB_B2 + (mo + 1) * 64)
                    nc.tensor.matmul(pt[:], bb[0:1, b2s],
                                     bb[0:1, BB_ONE:BB_ONE + B],
                                     start=False, stop=True)

                # ---- binning prep, per dh half -------------------------
                QT = [[stage.tile([64, B], BF16, name=f"QT{h}r{r}")
                       for r in range(4)] for h in range(2)]
                R1 = stage.tile([4, RHEAD], BF16, name="R1a")
                R2 = stage.tile([4, HELEMS - RHEAD], BF16, name="R2a")
                RAb = stage.tile([4, HELEMS], BF16, name="RAb")
                R3b = stage.tile([68, 11 * 512], BF16, name="R3b")
                R3a = stage.tile([68, 5 * 512], BF16, name="R3a")
                def flatten_row(hh, r):
                    if hh == 0:
                        # half a: head (36 partitions = 6 groups) first
                        # for an early z start, then the main piece
                        nc.sync.dma_start(R1[r:r + 1, :], QT[0][r][0:36, :])
                    else:
                        nc.sync.dma_start(RAb[r:r + 1, :], QT[1][r][0:64, :])

                def flatten_main(hh, r):
                    if hh == 0:
                        nc.sync.dma_start(R2[r:r + 1, :], QT[0][r][36:64, :])

                def flatten_scatter(hh):
                    # late-consumed pieces are scattered into per-row-group
                    # copies so their z matmuls run 3x packed: half a's
                    # main piece (walked last) and all of half b
                    if hh == 0:
                        rv = R2[:].rearrange("p (blk i) -> p blk i", i=512)
                        for h in range(3):
                            nblk = 4 if h == 2 else 5
                            nc.sync.dma_start(
                                R3a[32 * h:32 * h + 4, 0:nblk * 512],
                                rv[:, h:14:3, :])
                        return
                    rav = RAb[:].rearrange("p (blk i) -> p blk i", i=512)
                    for h in range(3):
                        nblk = 10 if h == 2 else 11
                        nc.sync.dma_start(
                            R3b[32 * h:32 * h + 4, 0:nblk * 512],
                            rav[:, h:32:3, :])

                warmn = [0]

                def warm_mm(dep_q, dep_r):
                    w = psA.tile([128, B], F32, tag="warm", bufs=1)
                    nc.tensor.matmul(w[:], QT[dep_q][dep_r][:, 0:128],
                                     QT[dep_q][dep_r][:, :],
                                     start=True, stop=True)
                    warmn[0] += 1

                inv = [None, None]

                def prep_half(hh):
                    lnm = work.tile([64, B], F32, tag=f"lnm{hh}")
                    nc.vector.tensor_tensor(lnm[:], po[2 + hh][:],
                                            f64[:, FO_MF + hh * B:FO_MF + hh * B + B],
                                            MULT)
                    ei = work.tile([64, B], F32, tag=f"ei{hh}")
                    nc.scalar.activation(ei[:], lnm[:], AF.Exp, bias=0.0,
                                         scale=-1.0)
                    iv = work.tile([64, B], F32, tag=f"inv{hh}")
                    inv[hh] = iv
                    nc.vector.tensor_tensor(
                        iv[:], ei[:], f64[:, FO_CE + hh * B:FO_CE + hh * B + B],
                        MULT)
                    nc.vector.tensor_copy(QT[hh][0][:], iv[:])      # ih
                    flatten_row(hh, 0)
                    nc.vector.tensor_tensor(QT[hh][1][:], iv[:],
                                            QT[hh][0][:], SUB)      # il
                    flatten_row(hh, 1)
                    warm_mm(hh, 0)
                    a4 = work.tile([64, B], F32, tag=f"a4{hh}")
                    nc.vector.tensor_tensor(
                        a4[:], f64[:, FO_RM + hh * B:FO_RM + hh * B + B],
                        po[hh][:], MULT)
                    mu_x = work.tile([64, B], F32, tag=f"mux{hh}")
                    nc.vector.tensor_tensor(mu_x[:], s[:, hh, :], a4[:], SUB)
                    mx = work.tile([64, B], F32, tag=f"mx{hh}")
                    nc.vector.tensor_tensor(mx[:], mu_x[:], iv[:], MULT)
                    nc.vector.tensor_copy(QT[hh][2][:], mx[:])      # hi
                    flatten_row(hh, 2)
                    nc.vector.tensor_tensor(QT[hh][3][:], mx[:],
                                            QT[hh][2][:], SUB)      # lo
                    flatten_row(hh, 3)
                    warm_mm(hh, 2)
                    for r in range(4):
                        flatten_main(hh, r)
                    flatten_scatter(hh)

                mm2_tile(2)
                mm2_tile(0)
                prep_half(0)
                mm2_tile(3)
                mm2_tile(1)
                prep_half(1)

            # ---- binning main loop -------------------------------------
            with (
                tc.tile_pool(name="psZ", bufs=2,
                             space=bass.MemorySpace.PSUM) as psZ,
                tc.tile_pool(name="psQ", bufs=1,
                             space=bass.MemorySpace.PSUM) as psQ,
                tc.tile_pool(name="erf", bufs=3) as epool,
            ):
                q = psQ.tile([128, B], F32)
                # (hh, base, gel) walk, z matmuls emitted one group ahead
                # of tanh/q so the PE never idles waiting on ACT.
                # Order: half-a heads (unpacked, available first), then
                # half b (packed), then half a's scattered tail (packed).
                walk = []
                for hh in range(2):
                    base = 0
                    for gel in GROUPS:
                        walk.append((hh, base, gel))
                        base += gel

                zts = {}

                def emit_z(gi):
                    hh, base, gel = walk[gi]
                    zt = psZ.tile([128, 1536], F32, tag="zt")
                    zts[gi] = zt
                    if hh == 0 and base < RHEAD:
                        for h in range(gel // 512):
                            off = base + h * 512
                            nc.tensor.matmul(
                                zt[:, h * 512:(h + 1) * 512], edg3[0:4, :],
                                R1[:, off:off + 512], start=True, stop=True)
                    else:
                        src_t = R3b if hh == 1 else R3a
                        g = base // 1536 if hh == 1 else (base - RHEAD) // 1536
                        for h in range(gel // 512):
                            nc.tensor.matmul(
                                zt[:, h * 512:(h + 1) * 512],
                                edg3[32 * h:32 * h + 4, :],
                                src_t[32 * h:32 * h + 4,
                                      g * 512:(g + 1) * 512],
                                start=True, stop=True,
                                tile_position=(32 * h, 0))

                emit_z(0)
                for gi, (hh, base, gel) in enumerate(walk):
                    if gi + 1 < len(walk):
                        emit_z(gi + 1)
                    zt = zts.pop(gi)
                    et = epool.tile([128, 1536], FP8, tag="et")
                    nc.scalar.activation(et[:, 0:gel], zt[:, 0:gel],
                                         AF.Tanh, bias=0.0, scale=ATAN)
                    for j in range(gel // 128):
                        c = (hh * HELEMS + base) // 128 + j
                        nc.tensor.matmul(q[:, c:c + 1],
                                         et[:, j * 128:(j + 1) * 128],
                                         uv[:], start=True, stop=True)

                # tail: part = sum_cols (sqw*(xqc - q))^2
                e1 = work.tile([128, B], F32)
                nc.vector.tensor_tensor(e1[:], f128[:, 0:B], q[:], SUB)
                dw = work.tile([128, B], F32)
                nc.vector.tensor_tensor(dw[:], e1[:], f128[:, B:2 * B], MULT)
                dw2 = work.tile([128, B], F32)
                part = work.tile([128, 1], F32)
                nc.vector.scalar_tensor_tensor(dw2[:], dw[:], 1.0, dw[:],
                                               op0=BYP, op1=MULT,
                                               accum_out=part[:])
                nc.sync.dma_start(d_part.ap()[:], part[:])

    nc.compile()
    return nc


def host_prep(x, t, noise, W1, b1, W2, b2):
    """Build the per-core in_maps (host-side packing + tiny per-row math)."""
    f32 = np.float32
    tv = t[:, 0].astype(f32)
    gamma = (1.0 - np.power(f32(SIGMA1), f32(2.0) * tv)).astype(f32)
    low = tv < TMIN
    mf = np.where(low, f32(0.0), f32(1.0)).astype(f32)
    gsafe = np.where(gamma > 0, gamma, f32(1.0)).astype(f32)
    r = np.sqrt((1.0 - gsafe) / gsafe).astype(f32)
    rsafe = np.where(r > 0, r, f32(1.0)).astype(f32)
    bv = ((1.0 - gamma) * mf).astype(f32)
    rm = (r * mf).astype(f32)
    cexp = np.where(low, f32(1.0 / np.sqrt(2.0)),
                    (1.0 / (rsafe * np.sqrt(2.0))).astype(f32)).astype(f32)
    sqw = np.power(f32(SIGMA1), -tv).astype(f32)

    xT = np.ascontiguousarray(x.T, dtype=f32)
    nT = np.ascontiguousarray(noise.T, dtype=f32)
    g2 = (gamma * (1.0 - gamma)).astype(f32)
    muT8 = np.ascontiguousarray(
        (xT * gamma[None, :] + nT * g2[None, :]).astype(f32)
        .reshape(8, 128, B).transpose(1, 0, 2).reshape(128, 8 * B)
        .astype(F8NP))

    # w1m[p, (m*8+k)*128 + c] = W1[k*128+p, m*128+c]
    w1f = W1[:D].astype(f32).reshape(8, 128, 16, 128)
    w1m = np.ascontiguousarray(
        w1f.transpose(1, 2, 0, 3).reshape(128, 16 * 8 * 128).astype(F8NP))

    # bb blob
    bbv = np.zeros((4, BB_W), dtype=BFNP)
    bbv[0, BB_TV:BB_TV + B] = tv.astype(BFNP)
    bbv[1, BB_TV:BB_TV + B] = BFNP(1.0)
    bbv[0, BB_W1T:BB_W1T + H] = W1[D].astype(BFNP)
    bbv[1, BB_W1T:BB_W1T + H] = b1.astype(BFNP)
    e = (2.0 * np.arange(1, K) / K - 1.0).astype(f32)  # 127 edges, bf16-exact
    bbv[0, BB_ONE:BB_ONE + B] = BFNP(1.0)
    edg3 = np.zeros((68, 128), dtype=BFNP)
    for hb in (0, 32, 64):
        edg3[hb + 0, :127] = e.astype(BFNP)
        edg3[hb + 1, :127] = e.astype(BFNP)
        edg3[hb + 2, :127] = BFNP(-1.0)
        edg3[hb + 3, :127] = BFNP(-1.0)

    # f64 blob (per-batch broadcasts, tiled x2 over dh)
    f64v = np.zeros((64, F64_W), dtype=f32)
    for off, v in ((FO_MF, mf), (FO_BV, bv), (FO_RM, rm), (FO_CE, cexp)):
        f64v[:, off:off + 512] = np.tile(v, 2)[None, :]

    uvec = np.zeros((128, 1), dtype=BFNP)
    uvec[:126, 0] = BFNP(-1.0 / K)
    uvec[126, 0] = BFNP(125.0 / 256.0)  # exact in bf16

    # q layout index math: q column c = hh*128 + r, partition p.
    # half a (hh=0) is linear: elem = r*128+p -> p64 = elem//256.
    # half b went through the 3-way block scatter: chunk k = r-12g,
    # h = k//4, i = (k%4)*128+p -> p64 = (3g+h)*2 + i//256.
    p_idx = np.arange(128)[:, None]
    c_idx = np.arange(B)[None, :]
    hh = c_idx // 128
    r = c_idx % 128
    lin = r * 128 + p_idx
    p64_a = lin // B
    b_a = lin % B
    gg = np.minimum(r // 12, 10)
    k = r - gg * 12
    i = (k % 4) * 128 + p_idx
    p64_b = (3 * gg + k // 4) * 2 + i // 256
    b_b = i % 256
    # half a: heads (q cols 0..71) linear; tail (cols 72..127) scattered
    gg_a = np.minimum((r - 72) // 12, 4)
    k_a = r - 72 - gg_a * 12
    i_a = (k_a % 4) * 128 + p_idx
    p64_at = 36 + (3 * gg_a + k_a // 4) * 2 + i_a // 256
    b_at = i_a % 256
    p64 = np.where(hh == 0, np.where(r < 72, p64_a, p64_at), p64_b)
    b_i = np.where(hh == 0, np.where(r < 72, b_a, b_at), b_b)
    d_l = hh * 64 + p64
    sqwq = np.ascontiguousarray(sqw[b_i], dtype=f32)

    def to64(a128):
        # [128 d, 256 b] -> [64 p, 2, 256] with [p, dh, b] = a[dh*64+p, b]
        return np.ascontiguousarray(
            a128.reshape(2, 64, B).transpose(1, 0, 2).reshape(64, 2 * B))

    in_maps = []
    for i in range(NCORES):
        cols = np.concatenate([np.arange(i * DSL, (i + 1) * DSL),
                               1024 + np.arange(i * DSL, (i + 1) * DSL)])
        # w2m[p, k*256 + c] = W2[k*128+p, cols[c]]
        w2m = np.ascontiguousarray(
            W2[:, cols].astype(f32).reshape(16, 128, 2 * DSL)
            .transpose(1, 0, 2).reshape(128, 16 * 2 * DSL).astype(F8NP))
        bbi = bbv.copy()
        bbi[0, BB_B2:BB_B2 + 2 * DSL] = b2[cols].astype(BFNP)
        f64i = f64v.copy()
        f64i[:, FO_XS:FO_XS + 512] = to64(xT[i * DSL:(i + 1) * DSL])
        f64i[:, FO_NS:FO_NS + 512] = to64(nT[i * DSL:(i + 1) * DSL])
        f128 = np.empty((128, 2 * B), dtype=f32)
        f128[:, 0:B] = x[b_i, i * DSL + d_l].astype(f32) + f32(C127)
        f128[:, B:2 * B] = sqwq
        in_maps.append({
            "muT8": muT8, "w1m": w1m, "w2m": w2m, "bb": bbi,
            "f64": f64i, "f128": f128, "uv": uvec, "edg3": edg3,
        })
    return in_maps


_nc_cache = {}


def get_nc(debug=False):
    if debug not in _nc_cache:
        _nc_cache[debug] = _build(debug)
    return _nc_cache[debug]


def run_on_cores(inputs, trace=False, debug=False, tmpdir=None):
    nc = get_nc(debug)
    in_maps = host_prep(**inputs)
    res = run_bass_kernel_spmd(nc, in_maps, core_ids=list(range(NCORES)),
                               trace=trace, tmpdir=tmpdir)
    total = np.float32(0.0)
    for i in range(NCORES):
        total += res.results[i]["part"].astype(np.float32).sum()
    loss = np.float32(-np.log(np.float32(SIGMA1)) * total / np.float32(B * D))
    return loss, res


def kernel(**inputs):
    inputs = {k: np.asarray(v) for k, v in inputs.items()}
    loss, _ = run_on_cores(inputs)
    return np.asarray(loss, dtype=np.float32)
